# revision 27
# baseline (speedup 1.0000x reference)
"""Trainium2 Bass kernel for nn_ModelConTT_46016279609475 (TT interpolation).

y[b] = v0[b]^T V1[b] V2[b] v3[b], where v_i are linearly-interpolated slices
of tiny TT cores at per-point grid coordinates derived from x[b, :].

Strategy (per NeuronCore, data-parallel over B):
  * Host precomputes two joint corner-packed tables (pure functions of the
    ~1MB cores, so no on-device table build or DRAM writeback):
      G[n0, n1, k] = sum_c core0[n0, c] * core1[c, n1, k]        (u-side)
      H[n2, n3, k] = sum_c core2[k, n2, c] * core3[c, n3]        (v-side)
    packed bf16 rows T[(a*128+b)] = [16 k x 4 corners] + 64 pad = 256B
    (dma_gather's minimum element), stacked G then H in ghd[32768, 128].
  * Per chunk ONE dma_gather fetches both sides (index position
    i = (s*jlen + j)*128 + p lands entry at dst[p, s*jlen + j, :], giving
    a [p, side, j, k, c] output directly).
  * Index lists are built on-device in dma_gather's wrapped layout
    (idx i at [i%16, i//16], replicated to rows 16-31 for queue 0's core
    pair) from a host-rearranged second copy of x (xq2), all on partition
    rows 0-15. H-side entries get +16384 by adding 128.0 to fl_d0 before
    the *128 combine.
  * Combine on DVE in bf16 (2x mode): m = g * W (corner weights bcast
    over k), pairwise corner adds, u_G * u_H, reduce over k into f32 y.
  * Software pipelining: idx chains run CHAIN_AHEAD chunks ahead of the
    gather stream and combines run LOOKAHEAD behind, so the gather DMA
    stream never waits on DVE; small first/last chunks shrink the
    pipeline fill/drain.
  * Exact-floor trick (f32-safe): t = (xc + 2^23) - 2^23 rounds to
    nearest; g = (t > xc); floor = t - g; frac = xc - floor is exact.

Batch mapping per core: shard b of size 32768; point i lives at
partition i%128, free col i//128 (y_pm[p, j] = y[j*128 + p]).
"""

import numpy as np
import ml_dtypes

import concourse.bass as bass
import concourse.bacc as bacc
import concourse.mybir as mybir
import concourse.tile as tile
from concourse import library_config
from concourse.bass_utils import run_bass_kernel_spmd

F32 = mybir.dt.float32
BF16 = mybir.dt.bfloat16
I16 = mybir.dt.int16
OP = mybir.AluOpType
AF = mybir.ActivationFunctionType

NCORES = 8
B = 262144
BS = B // NCORES          # 32768 points per core
P = 128                   # partitions
J = BS // P               # 256 free cols per partition
CHUNKS = (4, 4, 8, 16, 32, 32, 32, 32, 32, 32, 16, 8, 4, 4)  # j-cols per chunk
JMAX = max(CHUNKS)
LT = 2 * BS // 16         # 4096 idx-list cols total
N = 128                   # mode size
R = 16                    # TT rank
TE = N * N                # entries per table
EV = 64                   # useful values per entry: 16 k x 4 corners
ES = 128                  # stored row: EV values + pad to 256B
MAGIC = float(3 * 2 ** 22)   # 1.5*2^23: ulp-1 binade covers xc'-0.5 >= -0.5
SCALE = (N - 1) / 2.0     # 63.5
LOOKAHEAD = 1             # chunks the combines lag the gather stream
CHAIN_AHEAD = 4           # chunks the idx chains lead the gather stream

assert sum(CHUNKS) == J

_CACHED = None
DEBUG_TILES = {}


def _chunk_layout():
    """Per chunk: (jstart, jlen, list colstart)."""
    out = []
    jstart = 0
    for jlen in CHUNKS:
        out.append((jstart, jlen, 16 * jstart))
        jstart += jlen
    return out


def _build_nc():
    nc = bacc.Bacc("TRN2")

    x_pm = nc.dram_tensor("x_pm", [P, J, 4], F32, kind="ExternalInput")
    xq2 = nc.dram_tensor("xq2", [16, LT, 2], F32, kind="ExternalInput")
    ghd = nc.dram_tensor("ghd", [2 * TE, ES], BF16, kind="ExternalInput")
    y_pm = nc.dram_tensor("y_pm", [P, J], F32, kind="ExternalOutput")

    layout = _chunk_layout()
    nch = len(layout)

    with tile.TileContext(nc) as tc:
        with tc.tile_pool(name="per", bufs=1) as pe:
            nc.gpsimd.load_library(library_config.mlp)

            # idx list in dma_gather wrapped layout; rows 32+ only feed the
            # bounds check, memset once on Pool.
            LS = pe.tile([P, LT], I16)
            nc.gpsimd.memset(LS[:], 0)

            x_s = pe.tile([P, J * 4], F32)
            WGH = pe.tile([P, 2, J, 4], BF16)
            ysf = pe.tile([P, J], F32)
            xq2v = xq2[:].rearrange("p a b -> p (a b)")

            XQ = pe.tile([16, LT * 2], F32)

            def idx_chain(ch):
                """Build LS[:, cstart:cstart+lcc] on partition rows 0-15,
                in place on the preloaded XQ slice. floor(xc) computed as
                round-to-nearest(xc - 0.5) via the magic-add trick, all on
                Act: exact, and round-half-even at cell boundaries yields
                (fl, w=1) or (fl, w=0) - both interpolate correctly."""
                jstart, jlen, cstart = layout[ch]
                lcc = 16 * jlen
                fv = 2 * lcc                     # f32 values in this chunk
                a = XQ[:, 2 * cstart : 2 * cstart + fv]
                nc.scalar.activation(a, a, AF.Copy, bias=SCALE - 0.5, scale=SCALE)
                nc.scalar.activation(a, a, AF.Copy, bias=MAGIC, scale=1.0)
                nc.scalar.activation(a, a, AF.Copy, bias=-MAGIC, scale=1.0)
                f2v = a.rearrange("p (c d) -> p c d", d=2)
                # H-side (second half of the chunk's cols): fl_d0 += 128 so
                # idx = (fl_d0+128)*128 + fl_d1 lands in the H table rows.
                nc.vector.tensor_scalar(
                    f2v[:, lcc // 2 :, 0], f2v[:, lcc // 2 :, 0],
                    1.0, 128.0, OP.mult, OP.add,
                )
                nc.vector.scalar_tensor_tensor(
                    LS[0:16, cstart : cstart + lcc],
                    f2v[:, :, 0], 128.0, f2v[:, :, 1], OP.mult, OP.add,
                )
                nc.sync.dma_start(
                    LS[16:32, cstart : cstart + lcc],
                    LS[0:16, cstart : cstart + lcc],
                )

            def gather(ch, gth):
                _, jlen, cstart = layout[ch]
                ni = 2 * jlen * P
                nc.gpsimd.dma_gather(
                    gth[:, 0 : 2 * jlen, :],
                    ghd[:],
                    LS[:, cstart : cstart + 16 * jlen],
                    ni,
                    ni,
                    ES,
                    queue_num=0,
                    single_packet=False,
                )

            def weights_prep():
                # same Act-floor as the idx chains (engine-identical rounding
                # keeps fl consistent between the gather and weight paths)
                xc = pe.tile([P, J * 4], F32)
                nc.scalar.activation(
                    xc[:], x_s[:], AF.Copy, bias=SCALE - 0.5, scale=SCALE
                )
                fl = pe.tile([P, J * 4], F32)
                nc.scalar.activation(fl[:], xc[:], AF.Copy, bias=MAGIC, scale=1.0)
                nc.scalar.activation(fl[:], fl[:], AF.Copy, bias=-MAGIC, scale=1.0)
                # w = (xc' + 0.5) - fl ; a = 1 - w   (x_s, xc reused as outputs)
                wv = x_s
                nc.vector.scalar_tensor_tensor(
                    wv[:], xc[:], 0.5, fl[:], OP.add, OP.subtract
                )
                av = xc
                nc.vector.tensor_scalar(av[:], wv[:], -1.0, 1.0, OP.mult, OP.add)
                wvv = wv[:].rearrange("p (j d) -> p j d", d=4)
                avv = av[:].rearrange("p (j d) -> p j d", d=4)
                # corner weights; corner c=(dhi,dlo): c0=a*a, c1=a*w,
                # c2=w*a, c3=w*w over dims (0,1) for G and (2,3) for H
                for s, (d0, d1) in enumerate(((0, 1), (2, 3))):
                    for c, (h0, h1) in enumerate(((avv, avv), (avv, wvv),
                                                  (wvv, avv), (wvv, wvv))):
                        nc.vector.tensor_tensor(
                            WGH[:, s, :, c], h0[:, :, d0], h1[:, :, d1], OP.mult
                        )

            def combine(ch, gth, cb):
                jstart, jlen, _ = layout[ch]
                m = cb.tile([P, 2, JMAX, R, 4], BF16, tag="m")
                t2 = cb.tile([P, 2, JMAX, R, 2], BF16, tag="t2")
                u = cb.tile([P, 2, JMAX, R], BF16, tag="u")
                for s in range(2):
                    gv = gth[:, jlen * s : jlen * s + jlen, 0:EV].rearrange(
                        "p j (k c) -> p j k c", c=4
                    )
                    wb = (
                        WGH[:, s, jstart : jstart + jlen, :]
                        .unsqueeze(2)
                        .broadcast_to([P, jlen, R, 4])
                    )
                    nc.vector.tensor_tensor(m[:, s, 0:jlen], gv, wb, OP.mult)
                    nc.vector.tensor_tensor(
                        t2[:, s, 0:jlen],
                        m[:, s, 0:jlen, :, 0:2], m[:, s, 0:jlen, :, 2:4], OP.add,
                    )
                    nc.vector.tensor_tensor(
                        u[:, s, 0:jlen],
                        t2[:, s, 0:jlen, :, 0], t2[:, s, 0:jlen, :, 1], OP.add,
                    )
                pr = cb.tile([P, JMAX, R], BF16, tag="pr")
                nc.vector.tensor_tensor(
                    pr[:, 0:jlen], u[:, 0, 0:jlen], u[:, 1, 0:jlen], OP.mult
                )
                nc.vector.tensor_reduce(
                    ysf[:, jstart : jstart + jlen], pr[:, 0:jlen],
                    mybir.AxisListType.X, OP.add,
                )
                # y writeback issued from Act (idle by now) so SP's in-order
                # queue never blocks idx-chain work behind a pending combine;
                # the last chunks merge into one write to shrink the tail
                if ch < nch - 4:
                    nc.scalar.dma_start(
                        y_pm[:, jstart : jstart + jlen],
                        ysf[:, jstart : jstart + jlen],
                    )
                elif ch == nch - 1:
                    js4 = layout[nch - 4][0]
                    nc.scalar.dma_start(y_pm[:, js4:J], ysf[:, js4:J])

            with (
                tc.tile_pool(name="gbuf", bufs=5) as gb,
                tc.tile_pool(name="cbuf", bufs=2) as cb,
            ):
                # split xq2 load so chunk 0's chain starts after ~200ns
                head_v = 2 * 16 * sum(CHUNKS[:CHAIN_AHEAD])
                nc.sync.dma_start(XQ[:, 0:head_v], xq2v[:, 0:head_v])
                nc.sync.dma_start(XQ[:, head_v:], xq2v[:, head_v:])
                nc.sync.dma_start(x_s[:], x_pm[:].rearrange("p a b -> p (a b)"))
                for ch in range(CHAIN_AHEAD):
                    idx_chain(ch)
                weights_prep()

                gths = {}
                for ch in range(nch):
                    gth = gb.tile([P, 2 * JMAX, ES], BF16, tag="gth")
                    gather(ch, gth)
                    gths[ch] = gth
                    nxt = ch + CHAIN_AHEAD
                    if nxt < nch:
                        idx_chain(nxt)
                    if ch >= LOOKAHEAD:
                        combine(ch - LOOKAHEAD, gths.pop(ch - LOOKAHEAD), cb)
                for ch in range(nch - LOOKAHEAD, nch):
                    combine(ch, gths.pop(ch), cb)

            DEBUG_TILES.update(LS=LS, WGH=WGH, ysf=ysf)

    nc.finalize()
    return nc


def _make_tables(core0, core1, core2, core3):
    """Joint corner-packed bf16 tables, stacked G then H: [2*TE, ES]."""
    c0 = np.asarray(core0, dtype=np.float32)[0]        # [128, 16]
    c1 = np.asarray(core1, dtype=np.float32)           # [16, 128, 16]
    c2 = np.asarray(core2, dtype=np.float32)           # [16, 128, 16]
    c3 = np.asarray(core3, dtype=np.float32)[:, :, 0]  # [16, 128]

    G = np.einsum("ac,cbk->abk", c0, c1)               # [n0, n1, k]
    H = np.einsum("cae,eb->abc", c2, c3)               # [n2, n3, k]

    hi = np.minimum(np.arange(N) + 1, N - 1)

    def pack(T):
        # entry[(a*128+b), k, (dhi,dlo)] = T[a+dhi, b+dlo, k], padded to ES
        cs = np.stack([T, T[:, hi], T[hi], T[hi][:, hi]], axis=-1)
        out = np.zeros((TE, ES), dtype=np.float32)
        out[:, :EV] = cs.reshape(TE, EV)
        return out

    return np.concatenate([pack(G), pack(H)], axis=0).astype(ml_dtypes.bfloat16)


def _prep_inputs(x, core0, core1, core2, core3):
    """Shard x over cores; build the combine-layout copy (x_pm) and the
    wrapped idx-path copy (xq2); attach the shared host-built table."""
    xs = np.ascontiguousarray(np.asarray(x, dtype=np.float32).reshape(NCORES, BS, 4))
    ghd = _make_tables(core0, core1, core2, core3)

    # wrapped idx layout: within chunk ch, position i = (s*jlen + j)*128 + p,
    # global list col C = cstart + (s*jlen + j)*8 + p//16, row r = p%16.
    # xq2h[r, C, :] = x[b, (d0, d1)] for b = (jstart+j)*128+p,
    # dims (0,1) for s=0 and (2,3) for s=1.
    Cl, rl, bl, d0l = [], [], [], []
    jstart = 0
    for jlen in CHUNKS:
        cstart = 16 * jstart
        s_i, j_i, p_i = np.meshgrid(
            np.arange(2), np.arange(jlen), np.arange(P), indexing="ij"
        )
        Cl.append(cstart + (s_i * jlen + j_i) * 8 + p_i // 16)
        rl.append(p_i % 16)
        bl.append((jstart + j_i) * P + p_i)
        d0l.append(np.where(s_i == 0, 0, 2))
        jstart += jlen
    C = np.concatenate([a.ravel() for a in Cl])
    rr = np.concatenate([a.ravel() for a in rl])
    bb = np.concatenate([a.ravel() for a in bl])
    dd0 = np.concatenate([a.ravel() for a in d0l])

    in_maps = []
    for c in range(NCORES):
        xc_ = xs[c]
        x_pm = np.ascontiguousarray(
            xc_.reshape(J, P, 4).transpose(1, 0, 2)
        )  # [128, 256, 4]
        xq2h = np.empty((16, LT, 2), dtype=np.float32)
        xq2h[rr, C, 0] = xc_[bb, dd0]
        xq2h[rr, C, 1] = xc_[bb, dd0 + 1]
        in_maps.append({"x_pm": x_pm, "xq2": xq2h, "ghd": ghd})
    return in_maps


def kernel(x, core0, core1, core2, core3):
    global _CACHED
    if _CACHED is None:
        _CACHED = _build_nc()
    nc = _CACHED
    in_maps = _prep_inputs(x, core0, core1, core2, core3)
    res = run_bass_kernel_spmd(nc, in_maps, core_ids=list(range(NCORES)))
    outs = []
    for c in range(NCORES):
        y_pm = res.results[c]["y_pm"]          # [128, 256]
        outs.append(np.ascontiguousarray(np.asarray(y_pm).T).reshape(-1))
    return np.concatenate(outs).astype(np.float32)


# revision 28
# speedup vs baseline: 1.0184x; 1.0184x over previous
"""Trainium2 Bass kernel for nn_ModelConTT_46016279609475 (TT interpolation).

y[b] = v0[b]^T V1[b] V2[b] v3[b], where v_i are linearly-interpolated slices
of tiny TT cores at per-point grid coordinates derived from x[b, :].

Strategy (per NeuronCore, data-parallel over B):
  * Host precomputes two joint corner-packed tables (pure functions of the
    ~1MB cores, so no on-device table build or DRAM writeback):
      G[n0, n1, k] = sum_c core0[n0, c] * core1[c, n1, k]        (u-side)
      H[n2, n3, k] = sum_c core2[k, n2, c] * core3[c, n3]        (v-side)
    packed bf16 rows T[(a*128+b)] = [16 k x 4 corners] + 64 pad = 256B
    (dma_gather's minimum element), stacked G then H in ghd[32768, 128].
  * Per chunk ONE dma_gather fetches both sides (index position
    i = (s*jlen + j)*128 + p lands entry at dst[p, s*jlen + j, :], giving
    a [p, side, j, k, c] output directly).
  * Index lists are built on-device in dma_gather's wrapped layout
    (idx i at [i%16, i//16], replicated to rows 16-31 for queue 0's core
    pair) from a host-rearranged second copy of x (xq2), all on partition
    rows 0-15. H-side entries get +16384 by adding 128.0 to fl_d0 before
    the *128 combine.
  * Combine on DVE in bf16 (2x mode): m = g * W (corner weights bcast
    over k), pairwise corner adds, u_G * u_H, reduce over k into f32 y.
  * Software pipelining: idx chains run CHAIN_AHEAD chunks ahead of the
    gather stream and combines run LOOKAHEAD behind, so the gather DMA
    stream never waits on DVE; small first/last chunks shrink the
    pipeline fill/drain.
  * Exact-floor trick (f32-safe): t = (xc + 2^23) - 2^23 rounds to
    nearest; g = (t > xc); floor = t - g; frac = xc - floor is exact.

Batch mapping per core: shard b of size 32768; point i lives at
partition i%128, free col i//128 (y_pm[p, j] = y[j*128 + p]).
"""

import numpy as np
import ml_dtypes

import concourse.bass as bass
import concourse.bacc as bacc
import concourse.mybir as mybir
import concourse.tile as tile
from concourse import library_config
from concourse.bass_utils import run_bass_kernel_spmd

F32 = mybir.dt.float32
BF16 = mybir.dt.bfloat16
I16 = mybir.dt.int16
OP = mybir.AluOpType
AF = mybir.ActivationFunctionType

NCORES = 8
B = 262144
BS = B // NCORES          # 32768 points per core
P = 128                   # partitions
J = BS // P               # 256 free cols per partition
CHUNKS = (4, 4, 8, 16, 32, 32, 32, 32, 32, 32, 16, 8, 4, 4)  # j-cols per chunk
JMAX = max(CHUNKS)
LT = 2 * BS // 16         # 4096 idx-list cols total
N = 128                   # mode size
R = 16                    # TT rank
TE = N * N                # entries per table
EV = 64                   # useful values per entry: 16 k x 4 corners
ES = 128                  # stored row: EV values + pad to 256B
MAGIC = float(3 * 2 ** 22)   # 1.5*2^23: ulp-1 binade covers xc'-0.5 >= -0.5
SCALE = (N - 1) / 2.0     # 63.5
LOOKAHEAD = 1             # chunks the combines lag the gather stream
CHAIN_AHEAD = 4           # chunks the idx chains lead the gather stream

assert sum(CHUNKS) == J

_CACHED = None
DEBUG_TILES = {}


def _chunk_layout():
    """Per chunk: (jstart, jlen, list colstart)."""
    out = []
    jstart = 0
    for jlen in CHUNKS:
        out.append((jstart, jlen, 16 * jstart))
        jstart += jlen
    return out


def _build_nc():
    nc = bacc.Bacc("TRN2")

    x_pm = nc.dram_tensor("x_pm", [P, J, 4], F32, kind="ExternalInput")
    xq2 = nc.dram_tensor("xq2", [16, LT, 2], F32, kind="ExternalInput")
    ghd = nc.dram_tensor("ghd", [2 * TE, ES], BF16, kind="ExternalInput")
    y_pm = nc.dram_tensor("y_pm", [P, J], F32, kind="ExternalOutput")

    layout = _chunk_layout()
    nch = len(layout)

    with tile.TileContext(nc) as tc:
        with tc.tile_pool(name="per", bufs=1) as pe:
            nc.gpsimd.load_library(library_config.mlp)

            # idx list in dma_gather wrapped layout; rows 32+ only feed the
            # bounds check, memset once on Pool.
            LS = pe.tile([P, LT], I16)
            nc.gpsimd.memset(LS[:], 0)

            x_s = pe.tile([P, J * 4], F32)
            WGH = pe.tile([P, 2, J, 4], BF16)
            ysf = pe.tile([P, J], F32)
            xq2v = xq2[:].rearrange("p a b -> p (a b)")

            XQ = pe.tile([16, LT * 2], F32)

            def idx_chain(ch):
                """Build LS[:, cstart:cstart+lcc] on partition rows 0-15,
                in place on the preloaded XQ slice. floor(xc) computed as
                round-to-nearest(xc - 0.5) via the magic-add trick, all on
                Act: exact, and round-half-even at cell boundaries yields
                (fl, w=1) or (fl, w=0) - both interpolate correctly."""
                jstart, jlen, cstart = layout[ch]
                lcc = 16 * jlen
                fv = 2 * lcc                     # f32 values in this chunk
                a = XQ[:, 2 * cstart : 2 * cstart + fv]
                nc.scalar.activation(a, a, AF.Copy, bias=SCALE - 0.5, scale=SCALE)
                nc.scalar.activation(a, a, AF.Copy, bias=MAGIC, scale=1.0)
                nc.scalar.activation(a, a, AF.Copy, bias=-MAGIC, scale=1.0)
                f2v = a.rearrange("p (c d) -> p c d", d=2)
                # H-side (second half of the chunk's cols): fl_d0 += 128 so
                # idx = (fl_d0+128)*128 + fl_d1 lands in the H table rows.
                nc.vector.tensor_scalar(
                    f2v[:, lcc // 2 :, 0], f2v[:, lcc // 2 :, 0],
                    1.0, 128.0, OP.mult, OP.add,
                )
                nc.vector.scalar_tensor_tensor(
                    LS[0:16, cstart : cstart + lcc],
                    f2v[:, :, 0], 128.0, f2v[:, :, 1], OP.mult, OP.add,
                )
                nc.sync.dma_start(
                    LS[16:32, cstart : cstart + lcc],
                    LS[0:16, cstart : cstart + lcc],
                )

            def gather(ch, gth):
                _, jlen, cstart = layout[ch]
                ni = 2 * jlen * P
                nc.gpsimd.dma_gather(
                    gth[:, 0 : 2 * jlen, :],
                    ghd[:],
                    LS[:, cstart : cstart + 16 * jlen],
                    ni,
                    ni,
                    ES,
                    queue_num=0,
                    single_packet=False,
                )

            def weights_prep():
                # same Act-floor as the idx chains (engine-identical rounding
                # keeps fl consistent between the gather and weight paths)
                xc = pe.tile([P, J * 4], F32)
                nc.scalar.activation(
                    xc[:], x_s[:], AF.Copy, bias=SCALE - 0.5, scale=SCALE
                )
                fl = pe.tile([P, J * 4], F32)
                nc.scalar.activation(fl[:], xc[:], AF.Copy, bias=MAGIC, scale=1.0)
                nc.scalar.activation(fl[:], fl[:], AF.Copy, bias=-MAGIC, scale=1.0)
                # w = (xc' + 0.5) - fl ; a = 1 - w   (x_s, xc reused as outputs)
                wv = x_s
                nc.vector.scalar_tensor_tensor(
                    wv[:], xc[:], 0.5, fl[:], OP.add, OP.subtract
                )
                av = xc
                nc.vector.tensor_scalar(av[:], wv[:], -1.0, 1.0, OP.mult, OP.add)
                wvv = wv[:].rearrange("p (j d) -> p j d", d=4)
                avv = av[:].rearrange("p (j d) -> p j d", d=4)
                # corner weights; corner c=(dhi,dlo): c0=a*a, c1=a*w,
                # c2=w*a, c3=w*w over dims (0,1) for G and (2,3) for H
                for s, (d0, d1) in enumerate(((0, 1), (2, 3))):
                    for c, (h0, h1) in enumerate(((avv, avv), (avv, wvv),
                                                  (wvv, avv), (wvv, wvv))):
                        nc.vector.tensor_tensor(
                            WGH[:, s, :, c], h0[:, :, d0], h1[:, :, d1], OP.mult
                        )

            def combine(ch, gth, cb):
                jstart, jlen, _ = layout[ch]
                m = cb.tile([P, 2, JMAX, R, 4], BF16, tag="m")
                t2 = cb.tile([P, 2, JMAX, R, 2], BF16, tag="t2")
                u = cb.tile([P, 2, JMAX, R], BF16, tag="u")
                for s in range(2):
                    gv = gth[:, jlen * s : jlen * s + jlen, 0:EV].rearrange(
                        "p j (k c) -> p j k c", c=4
                    )
                    wb = (
                        WGH[:, s, jstart : jstart + jlen, :]
                        .unsqueeze(2)
                        .broadcast_to([P, jlen, R, 4])
                    )
                    nc.vector.tensor_tensor(m[:, s, 0:jlen], gv, wb, OP.mult)
                    nc.vector.tensor_tensor(
                        t2[:, s, 0:jlen],
                        m[:, s, 0:jlen, :, 0:2], m[:, s, 0:jlen, :, 2:4], OP.add,
                    )
                    nc.vector.tensor_tensor(
                        u[:, s, 0:jlen],
                        t2[:, s, 0:jlen, :, 0], t2[:, s, 0:jlen, :, 1], OP.add,
                    )
                pr = cb.tile([P, JMAX, R], BF16, tag="pr")
                nc.vector.tensor_tensor(
                    pr[:, 0:jlen], u[:, 0, 0:jlen], u[:, 1, 0:jlen], OP.mult
                )
                nc.vector.tensor_reduce(
                    ysf[:, jstart : jstart + jlen], pr[:, 0:jlen],
                    mybir.AxisListType.X, OP.add,
                )
                # y writeback issued from Act (idle by now) so SP's in-order
                # queue never blocks idx-chain work behind a pending combine;
                # the last chunks merge into one write to shrink the tail
                if ch < nch - 4:
                    nc.scalar.dma_start(
                        y_pm[:, jstart : jstart + jlen],
                        ysf[:, jstart : jstart + jlen],
                    )
                elif ch == nch - 1:
                    js4 = layout[nch - 4][0]
                    nc.scalar.dma_start(y_pm[:, js4:J], ysf[:, js4:J])

            with (
                tc.tile_pool(name="gbuf", bufs=4) as gb,
                tc.tile_pool(name="cbuf", bufs=2) as cb,
            ):
                # split xq2 load so chunk 0's chain starts after ~200ns
                head_v = 2 * 16 * sum(CHUNKS[:CHAIN_AHEAD])
                nc.sync.dma_start(XQ[:, 0:head_v], xq2v[:, 0:head_v])
                nc.sync.dma_start(XQ[:, head_v:], xq2v[:, head_v:])
                nc.sync.dma_start(x_s[:], x_pm[:].rearrange("p a b -> p (a b)"))
                for ch in range(CHAIN_AHEAD):
                    idx_chain(ch)
                weights_prep()

                gths = {}
                for ch in range(nch):
                    gth = gb.tile([P, 2 * JMAX, ES], BF16, tag="gth")
                    gather(ch, gth)
                    gths[ch] = gth
                    nxt = ch + CHAIN_AHEAD
                    if nxt < nch:
                        idx_chain(nxt)
                    if ch >= LOOKAHEAD:
                        combine(ch - LOOKAHEAD, gths.pop(ch - LOOKAHEAD), cb)
                for ch in range(nch - LOOKAHEAD, nch):
                    combine(ch, gths.pop(ch), cb)

            DEBUG_TILES.update(LS=LS, WGH=WGH, ysf=ysf)

    nc.finalize()
    return nc


def _make_tables(core0, core1, core2, core3):
    """Joint corner-packed bf16 tables, stacked G then H: [2*TE, ES]."""
    c0 = np.asarray(core0, dtype=np.float32)[0]        # [128, 16]
    c1 = np.asarray(core1, dtype=np.float32)           # [16, 128, 16]
    c2 = np.asarray(core2, dtype=np.float32)           # [16, 128, 16]
    c3 = np.asarray(core3, dtype=np.float32)[:, :, 0]  # [16, 128]

    G = np.einsum("ac,cbk->abk", c0, c1)               # [n0, n1, k]
    H = np.einsum("cae,eb->abc", c2, c3)               # [n2, n3, k]

    hi = np.minimum(np.arange(N) + 1, N - 1)

    def pack(T):
        # entry[(a*128+b), k, (dhi,dlo)] = T[a+dhi, b+dlo, k], padded to ES
        cs = np.stack([T, T[:, hi], T[hi], T[hi][:, hi]], axis=-1)
        out = np.zeros((TE, ES), dtype=np.float32)
        out[:, :EV] = cs.reshape(TE, EV)
        return out

    return np.concatenate([pack(G), pack(H)], axis=0).astype(ml_dtypes.bfloat16)


def _prep_inputs(x, core0, core1, core2, core3):
    """Shard x over cores; build the combine-layout copy (x_pm) and the
    wrapped idx-path copy (xq2); attach the shared host-built table."""
    xs = np.ascontiguousarray(np.asarray(x, dtype=np.float32).reshape(NCORES, BS, 4))
    ghd = _make_tables(core0, core1, core2, core3)

    # wrapped idx layout: within chunk ch, position i = (s*jlen + j)*128 + p,
    # global list col C = cstart + (s*jlen + j)*8 + p//16, row r = p%16.
    # xq2h[r, C, :] = x[b, (d0, d1)] for b = (jstart+j)*128+p,
    # dims (0,1) for s=0 and (2,3) for s=1.
    Cl, rl, bl, d0l = [], [], [], []
    jstart = 0
    for jlen in CHUNKS:
        cstart = 16 * jstart
        s_i, j_i, p_i = np.meshgrid(
            np.arange(2), np.arange(jlen), np.arange(P), indexing="ij"
        )
        Cl.append(cstart + (s_i * jlen + j_i) * 8 + p_i // 16)
        rl.append(p_i % 16)
        bl.append((jstart + j_i) * P + p_i)
        d0l.append(np.where(s_i == 0, 0, 2))
        jstart += jlen
    C = np.concatenate([a.ravel() for a in Cl])
    rr = np.concatenate([a.ravel() for a in rl])
    bb = np.concatenate([a.ravel() for a in bl])
    dd0 = np.concatenate([a.ravel() for a in d0l])

    in_maps = []
    for c in range(NCORES):
        xc_ = xs[c]
        x_pm = np.ascontiguousarray(
            xc_.reshape(J, P, 4).transpose(1, 0, 2)
        )  # [128, 256, 4]
        xq2h = np.empty((16, LT, 2), dtype=np.float32)
        xq2h[rr, C, 0] = xc_[bb, dd0]
        xq2h[rr, C, 1] = xc_[bb, dd0 + 1]
        in_maps.append({"x_pm": x_pm, "xq2": xq2h, "ghd": ghd})
    return in_maps


def kernel(x, core0, core1, core2, core3):
    global _CACHED
    if _CACHED is None:
        _CACHED = _build_nc()
    nc = _CACHED
    in_maps = _prep_inputs(x, core0, core1, core2, core3)
    res = run_bass_kernel_spmd(nc, in_maps, core_ids=list(range(NCORES)))
    outs = []
    for c in range(NCORES):
        y_pm = res.results[c]["y_pm"]          # [128, 256]
        outs.append(np.ascontiguousarray(np.asarray(y_pm).T).reshape(-1))
    return np.concatenate(outs).astype(np.float32)


# revision 38
# speedup vs baseline: 1.1438x; 1.1231x over previous
"""Trainium2 Bass kernel for nn_ModelConTT_46016279609475 (TT interpolation).

y[b] = v0[b]^T V1[b] V2[b] v3[b], where v_i are linearly-interpolated slices
of tiny TT cores at per-point grid coordinates derived from x[b, :].

Strategy (per NeuronCore, data-parallel over B):
  * Host precomputes two joint corner-packed tables (pure functions of the
    ~1MB cores, so no on-device table build or DRAM writeback):
      G[n0, n1, k] = sum_c core0[n0, c] * core1[c, n1, k]        (u-side)
      H[n2, n3, k] = sum_c core2[k, n2, c] * core3[c, n3]        (v-side)
    packed bf16 rows T[(a*128+b)] = [16 k x 4 corners] + 64 pad = 256B
    (dma_gather's minimum element), stacked G then H in ghd[32768, 128].
  * Per chunk ONE dma_gather fetches both sides (index position
    i = (s*jlen + j)*128 + p lands entry at dst[p, s*jlen + j, :], giving
    a [p, side, j, k, c] output directly).
  * Index lists are built on-device in dma_gather's wrapped layout
    (idx i at [i%16, i//16]; queue 0's Q7 core pair reads rows 0-31) from
    a host-rearranged second copy of x (xq2) whose rows 16-31 duplicate
    0-15, so one 32-row STT writes the whole list - no replicate DMA.
    H-side entries get +16384 by adding 128.0 to fl_d0 before the *128
    combine.
  * Combine on DVE in bf16 (2x mode): m = g * W (corner weights bcast
    over k), pairwise corner adds, u_G * u_H, reduce over k into f32 y.
  * Software pipelining: idx chains run CHAIN_AHEAD chunks ahead of the
    gather stream and combines run LOOKAHEAD behind, so the gather DMA
    stream never waits on DVE; small first/last chunks shrink the
    pipeline fill/drain.
  * Floor on the Act engine only: fl = round(xc - 0.5) via the magic-add
    trick at 1.5*2^23 (ulp-1 binade covers the near-zero negatives).
    Round-half-even at cell boundaries yields (fl, w=1) or (fl, w=0),
    both of which interpolate correctly; fl is in [0, 126] for x in
    [-1, 1). The weight path computes fl with the bitwise-identical op
    sequence so gathered cells and weights always agree.

Batch mapping per core: shard b of size 32768; point i lives at
partition i%128, free col i//128 (y_pm[p, j] = y[j*128 + p]).
"""

import numpy as np
import ml_dtypes

import concourse.bass as bass
import concourse.bacc as bacc
import concourse.mybir as mybir
import concourse.tile as tile
from concourse import library_config
from concourse.bass_utils import run_bass_kernel_spmd

F32 = mybir.dt.float32
BF16 = mybir.dt.bfloat16
I16 = mybir.dt.int16
OP = mybir.AluOpType
AF = mybir.ActivationFunctionType

NCORES = 8
B = 262144
BS = B // NCORES          # 32768 points per core
P = 128                   # partitions
J = BS // P               # 256 free cols per partition
CHUNKS = (4, 4, 8, 16, 32, 32, 32, 32, 32, 32, 16, 8, 8)  # j-cols per chunk
JMAX = max(CHUNKS)
LT = 2 * BS // 16         # 4096 idx-list cols total
N = 128                   # mode size
R = 16                    # TT rank
TE = N * N                # entries per table
EV = 64                   # useful values per entry: 16 k x 4 corners
ES = 128                  # stored row: EV values + pad to 256B
MAGIC = float(3 * 2 ** 22)   # 1.5*2^23: ulp-1 binade covers xc'-0.5 >= -0.5
SCALE = (N - 1) / 2.0     # 63.5
LOOKAHEAD = 1             # chunks the combines lag the gather stream
CHAIN_AHEAD = 3           # chunks the idx chains lead the gather stream

assert sum(CHUNKS) == J

_CACHED = None
DEBUG_TILES = {}


def _chunk_layout():
    """Per chunk: (jstart, jlen, list colstart)."""
    out = []
    jstart = 0
    for jlen in CHUNKS:
        out.append((jstart, jlen, 16 * jstart))
        jstart += jlen
    return out


def _build_nc():
    nc = bacc.Bacc("TRN2")

    x_pm = nc.dram_tensor("x_pm", [P, J, 4], F32, kind="ExternalInput")
    xq2 = nc.dram_tensor("xq2", [32, LT, 2], F32, kind="ExternalInput")
    ghd = nc.dram_tensor("ghd", [2 * TE, ES], BF16, kind="ExternalInput")
    y_pm = nc.dram_tensor("y_pm", [P, J], F32, kind="ExternalOutput")

    layout = _chunk_layout()
    nch = len(layout)

    with tile.TileContext(nc) as tc:
        with tc.tile_pool(name="per", bufs=1) as pe:
            nc.gpsimd.load_library(library_config.mlp)

            # idx list in dma_gather wrapped layout; rows 32+ only feed the
            # bounds check, memset once on Pool.
            LS = pe.tile([P, LT], I16)
            nc.gpsimd.memset(LS[:], 0)

            x_s = pe.tile([P, J * 4], F32)
            WGH = pe.tile([P, 2, J, 4], BF16)
            ysf = pe.tile([P, J], F32)
            xq2v = xq2[:].rearrange("p a b -> p (a b)")

            XQ = pe.tile([32, LT * 2], F32)

            def idx_chain(ch):
                """Build LS[:, cstart:cstart+lcc] on partition rows 0-31,
                in place on the preloaded XQ slice. floor(xc) computed as
                round-to-nearest(xc - 0.5) via the magic-add trick, all on
                Act: exact, and round-half-even at cell boundaries yields
                (fl, w=1) or (fl, w=0) - both interpolate correctly."""
                jstart, jlen, cstart = layout[ch]
                lcc = 16 * jlen
                fv = 2 * lcc                     # f32 values in this chunk
                a = XQ[:, 2 * cstart : 2 * cstart + fv]
                nc.scalar.activation(a, a, AF.Copy, bias=SCALE - 0.5, scale=SCALE)
                nc.scalar.activation(a, a, AF.Copy, bias=MAGIC, scale=1.0)
                nc.scalar.activation(a, a, AF.Copy, bias=-MAGIC, scale=1.0)
                f2v = a.rearrange("p (c d) -> p c d", d=2)
                # H-side (second half of the chunk's cols): fl_d0 += 128 so
                # idx = (fl_d0+128)*128 + fl_d1 lands in the H table rows.
                nc.vector.tensor_scalar(
                    f2v[:, lcc // 2 :, 0], f2v[:, lcc // 2 :, 0],
                    1.0, 128.0, OP.mult, OP.add,
                )
                nc.vector.scalar_tensor_tensor(
                    LS[0:32, cstart : cstart + lcc],
                    f2v[:, :, 0], 128.0, f2v[:, :, 1], OP.mult, OP.add,
                )

            def gather(ch, gth):
                _, jlen, cstart = layout[ch]
                ni = 2 * jlen * P
                nc.gpsimd.dma_gather(
                    gth[:, 0 : 2 * jlen, :],
                    ghd[:],
                    LS[:, cstart : cstart + 16 * jlen],
                    ni,
                    ni,
                    ES,
                    queue_num=0,
                    single_packet=False,
                )

            def weights_prep():
                # same Act-floor as the idx chains (engine-identical rounding
                # keeps fl consistent between the gather and weight paths)
                xc = pe.tile([P, J * 4], F32)
                nc.scalar.activation(
                    xc[:], x_s[:], AF.Copy, bias=SCALE - 0.5, scale=SCALE
                )
                # same fused op sequence as the idx chains: fl bitwise-equal
                fl = pe.tile([P, J * 4], F32)
                nc.scalar.activation(
                    fl[:], x_s[:], AF.Copy, bias=SCALE - 0.5 + MAGIC, scale=SCALE
                )
                nc.scalar.activation(fl[:], fl[:], AF.Copy, bias=-MAGIC, scale=1.0)
                # w = (xc' + 0.5) - fl ; a = 1 - w   (x_s, xc reused as outputs)
                wv = x_s
                nc.vector.scalar_tensor_tensor(
                    wv[:], xc[:], 0.5, fl[:], OP.add, OP.subtract
                )
                av = xc
                nc.vector.tensor_scalar(av[:], wv[:], -1.0, 1.0, OP.mult, OP.add)
                wvv = wv[:].rearrange("p (j d) -> p j d", d=4)
                avv = av[:].rearrange("p (j d) -> p j d", d=4)
                # corner weights; corner c=(dhi,dlo): c0=a*a, c1=a*w,
                # c2=w*a, c3=w*w over dims (0,1) for G and (2,3) for H
                for s, (d0, d1) in enumerate(((0, 1), (2, 3))):
                    for c, (h0, h1) in enumerate(((avv, avv), (avv, wvv),
                                                  (wvv, avv), (wvv, wvv))):
                        nc.vector.tensor_tensor(
                            WGH[:, s, :, c], h0[:, :, d0], h1[:, :, d1], OP.mult
                        )

            def combine(ch, gth, cb):
                jstart, jlen, _ = layout[ch]
                m = cb.tile([P, 2, JMAX, R, 4], BF16, tag="m")
                t2 = cb.tile([P, 2, JMAX, R, 2], BF16, tag="t2")
                u = cb.tile([P, 2, JMAX, R], BF16, tag="u")
                for s in range(2):
                    gv = gth[:, jlen * s : jlen * s + jlen, 0:EV].rearrange(
                        "p j (k c) -> p j k c", c=4
                    )
                    wb = (
                        WGH[:, s, jstart : jstart + jlen, :]
                        .unsqueeze(2)
                        .broadcast_to([P, jlen, R, 4])
                    )
                    nc.vector.tensor_tensor(m[:, s, 0:jlen], gv, wb, OP.mult)
                    nc.vector.tensor_tensor(
                        t2[:, s, 0:jlen],
                        m[:, s, 0:jlen, :, 0:2], m[:, s, 0:jlen, :, 2:4], OP.add,
                    )
                    nc.vector.tensor_tensor(
                        u[:, s, 0:jlen],
                        t2[:, s, 0:jlen, :, 0], t2[:, s, 0:jlen, :, 1], OP.add,
                    )
                pr = cb.tile([P, JMAX, R], BF16, tag="pr")
                nc.vector.tensor_tensor(
                    pr[:, 0:jlen], u[:, 0, 0:jlen], u[:, 1, 0:jlen], OP.mult
                )
                nc.vector.tensor_reduce(
                    ysf[:, jstart : jstart + jlen], pr[:, 0:jlen],
                    mybir.AxisListType.X, OP.add,
                )
                # y writeback issued from Act (idle by now) so SP's in-order
                # queue never blocks idx-chain work behind a pending combine;
                # the last chunks merge into one write to shrink the tail
                if ch < nch - 4:
                    nc.scalar.dma_start(
                        y_pm[:, jstart : jstart + jlen],
                        ysf[:, jstart : jstart + jlen],
                    )
                elif ch == nch - 1:
                    js4 = layout[nch - 4][0]
                    nc.scalar.dma_start(y_pm[:, js4:J], ysf[:, js4:J])

            with (
                tc.tile_pool(name="gbuf", bufs=4) as gb,
                tc.tile_pool(name="cbuf", bufs=2) as cb,
            ):
                # split xq2 load so chunk 0's chain starts after ~200ns
                head_v = 2 * 16 * sum(CHUNKS[:CHAIN_AHEAD])
                nc.sync.dma_start(XQ[:, 0 : 2 * head_v], xq2v[:, 0 : 2 * head_v])
                nc.sync.dma_start(XQ[:, 2 * head_v :], xq2v[:, 2 * head_v :])
                nc.sync.dma_start(x_s[:], x_pm[:].rearrange("p a b -> p (a b)"))
                for ch in range(CHAIN_AHEAD):
                    idx_chain(ch)
                weights_prep()

                gths = {}
                for ch in range(nch):
                    gth = gb.tile([P, 2 * JMAX, ES], BF16, tag="gth")
                    gather(ch, gth)
                    gths[ch] = gth
                    if ch < 6:
                        nxt = ch + CHAIN_AHEAD
                        if nxt < nch:
                            idx_chain(nxt)
                    elif ch == 6:
                        for nxt in range(6 + CHAIN_AHEAD, nch):
                            idx_chain(nxt)
                    if ch >= LOOKAHEAD:
                        combine(ch - LOOKAHEAD, gths.pop(ch - LOOKAHEAD), cb)
                for ch in range(nch - LOOKAHEAD, nch):
                    combine(ch, gths.pop(ch), cb)

            DEBUG_TILES.update(LS=LS, WGH=WGH, ysf=ysf)

    nc.finalize()
    return nc


def _make_tables(core0, core1, core2, core3):
    """Joint corner-packed bf16 tables, stacked G then H: [2*TE, ES]."""
    c0 = np.asarray(core0, dtype=np.float32)[0]        # [128, 16]
    c1 = np.asarray(core1, dtype=np.float32)           # [16, 128, 16]
    c2 = np.asarray(core2, dtype=np.float32)           # [16, 128, 16]
    c3 = np.asarray(core3, dtype=np.float32)[:, :, 0]  # [16, 128]

    G = np.einsum("ac,cbk->abk", c0, c1)               # [n0, n1, k]
    H = np.einsum("cae,eb->abc", c2, c3)               # [n2, n3, k]

    hi = np.minimum(np.arange(N) + 1, N - 1)

    def pack(T):
        # entry[(a*128+b), k, (dhi,dlo)] = T[a+dhi, b+dlo, k], padded to ES
        cs = np.stack([T, T[:, hi], T[hi], T[hi][:, hi]], axis=-1)
        out = np.zeros((TE, ES), dtype=np.float32)
        out[:, :EV] = cs.reshape(TE, EV)
        return out

    return np.concatenate([pack(G), pack(H)], axis=0).astype(ml_dtypes.bfloat16)


def _prep_inputs(x, core0, core1, core2, core3):
    """Shard x over cores; build the combine-layout copy (x_pm) and the
    wrapped idx-path copy (xq2); attach the shared host-built table."""
    xs = np.ascontiguousarray(np.asarray(x, dtype=np.float32).reshape(NCORES, BS, 4))
    ghd = _make_tables(core0, core1, core2, core3)

    # wrapped idx layout: within chunk ch, position i = (s*jlen + j)*128 + p,
    # global list col C = cstart + (s*jlen + j)*8 + p//16, row r = p%16.
    # xq2h[r, C, :] = x[b, (d0, d1)] for b = (jstart+j)*128+p,
    # dims (0,1) for s=0 and (2,3) for s=1.
    Cl, rl, bl, d0l = [], [], [], []
    jstart = 0
    for jlen in CHUNKS:
        cstart = 16 * jstart
        s_i, j_i, p_i = np.meshgrid(
            np.arange(2), np.arange(jlen), np.arange(P), indexing="ij"
        )
        Cl.append(cstart + (s_i * jlen + j_i) * 8 + p_i // 16)
        rl.append(p_i % 16)
        bl.append((jstart + j_i) * P + p_i)
        d0l.append(np.where(s_i == 0, 0, 2))
        jstart += jlen
    C = np.concatenate([a.ravel() for a in Cl])
    rr = np.concatenate([a.ravel() for a in rl])
    bb = np.concatenate([a.ravel() for a in bl])
    dd0 = np.concatenate([a.ravel() for a in d0l])

    in_maps = []
    for c in range(NCORES):
        xc_ = xs[c]
        x_pm = np.ascontiguousarray(
            xc_.reshape(J, P, 4).transpose(1, 0, 2)
        )  # [128, 256, 4]
        xq2h = np.empty((32, LT, 2), dtype=np.float32)
        xq2h[rr, C, 0] = xc_[bb, dd0]
        xq2h[rr, C, 1] = xc_[bb, dd0 + 1]
        xq2h[16:32] = xq2h[0:16]
        in_maps.append({"x_pm": x_pm, "xq2": xq2h, "ghd": ghd})
    return in_maps


def kernel(x, core0, core1, core2, core3):
    global _CACHED
    if _CACHED is None:
        _CACHED = _build_nc()
    nc = _CACHED
    in_maps = _prep_inputs(x, core0, core1, core2, core3)
    res = run_bass_kernel_spmd(nc, in_maps, core_ids=list(range(NCORES)))
    outs = []
    for c in range(NCORES):
        y_pm = res.results[c]["y_pm"]          # [128, 256]
        outs.append(np.ascontiguousarray(np.asarray(y_pm).T).reshape(-1))
    return np.concatenate(outs).astype(np.float32)
